# revision 63
# baseline (speedup 1.0000x reference)
"""Trainium2 Bass kernel for windowed mean-pooling (segment_reduce).

Computes, for each (batch b, window w):
    out[b, w, :] = mean over t in [begins[b,w], ends'[b,w]) of features[b, t, :]
where ends' = clip(ends, begins, begins + 8) (the reference gathers at most
MAX_WINDOW=8 tokens) and empty windows produce 0 (count clamped to >= 1).

Strategy (data-parallel over batch, one sample per NeuronCore):
  - Windows are mean-pooled via mask matmuls: out_slot = M^T @ F with M a
    host-built fp8 0/1 strip, F the slot's tokens, fp32 PSUM accumulate.
  - Windows are assigned to SLOTS greedily in sorted-begin order: a slot
    takes up to 128 windows as long as the union of their token intervals
    fits in 256 tokens.  The slot's tokens are RE-PACKED on host into
    exactly 2 aligned K-tiles, so every slot costs exactly 2 (slot, K-tile)
    mask matmul pairs (~33 total vs ~49 for global aligned packing); slots
    whose tokens fit one tile (the runt) cost a single pair.
  - The kernel is HBM-byte-bound (~5.3 MB/core ~ 15 us at 358 GB/s), so
    bytes are minimized hard: features fp8 E3M4 (~3.2 MB incl. repack
    padding), masks fp8 with the tile-0 strip's zero column-tail trimmed
    (the full-width tile-1 strip runs FIRST so its start=True write
    has_written-covers every PSUM row; the trimmed strip then accumulates),
    outputs fp8 in a partition-major [P, ns*D] layout (contiguous multi-KB
    descriptors).
  - ALL inputs ship as ONE unified fp8 slab: each slot's block holds its
    two feature tiles followed by its mask strips, with the fp32 1/count
    block once at the head (read back via 4-byte fp8->fp32 bitcasts).
    Every dma_start pays a ~2 us HBM completion receipt before its sem
    fires, so the input side is just 6 slot-grouped chunk DMAs (sizes
    [1,2,3,4,4,3] slots, GPSIMD SWDGE) -- each slot's matmuls gate on a
    SINGLE sem, and the SP HWDGE ring carries only the output drains.
    Merging the separate iv / mask / feature streams this way measured
    ~2 us + ~1.3 us faster than the best split-stream schedules.
  - PSUM evacuation applies the per-window 1/count scale, split 384+384
    across ScalarE ((N+352)/1.2 ns) and VectorE ((N+~210)/0.96 ns), which
    balance at that split.  PSUM tiles are [P, 1024] = exactly 2 banks so
    no two in-flight slots share a bank (a shared bank serializes the DVE
    read of slot s against slot s+1's PE writes); 4 bufs fill all 8 banks.
  - Startup: warm-up matmuls hold the PE's HAM activity window open while
    the first chunk lands; the warm-up source comes from a VectorE memset
    so GpSimd's first op is SWDGE descriptor generation.
"""

import os
import sys

import numpy as np

for _p in ("/opt/trn_rl_repo", "/root/.axon_site/_ro/trn_rl_repo"):
    if os.path.isdir(_p) and _p not in sys.path:
        sys.path.insert(0, _p)

from concourse import bacc, mybir  # noqa: E402
import concourse.tile as tile  # noqa: E402
from concourse.bass_utils import run_bass_kernel_spmd  # noqa: E402

B, T, D, W = 8, 4096, 768, 2048
MAXWIN = 8
P = 128
SLOT_TOK = 256  # tokens per slot (2 K-tiles, repacked)
N_WARM = 5  # PE p-state warm-up matmuls bridging until the first DMAs land
IVSLOT = 0  # the fp32 1/count block sits at the head of the unified slab
# so it arrives with the first chunk (the first evacuation needs it)
F32 = mybir.dt.float32
FP8 = mybir.dt.float8e3
NP_FP8 = mybir.dt.np(mybir.dt.float8e3)


def _fchunks(n, sizes0=(2, 4, 6), big=8):
    """Chunk sizes, small first so early slots' data lands first."""
    sizes = []
    for s in sizes0:
        if sum(sizes) + s > n:
            break
        sizes.append(s)
    rem = n - sum(sizes)
    while rem > 0:
        take = min(big, rem)
        sizes.append(take)
        rem -= take
    assert sum(sizes) == n and all(s > 0 for s in sizes), (sizes, n)
    return sizes


def _ogroups(ns):
    """Output DMA slot groups; small tail groups shorten the drain."""
    groups = []
    rem = ns
    while rem > 3:
        take = min(5, rem - 2)
        groups.append(take)
        rem -= take
    if rem > 1:
        groups.append(rem - 1)
        rem = 1
    groups.append(1)
    assert sum(groups) == ns
    return groups


def _build_program(ns, pairs, total, ivbase, slotcols, last_rows=P):
    """pairs: per slot, list of (mask col base, width, psum row offset,
    feature col base) into ONE unified slab that interleaves each slot's
    feature tiles, mask strips, and (once) the fp32 1/count block -- so a
    slot's matmuls gate on a single chunk-DMA completion sem, and the whole
    input side is 6 dma_starts (each completion receipt costs ~2 us).
    slotcols: per-slot start column (len ns+1).  last_rows: used rows of
    the final (runt) slot -- its out DMA ships only those partitions."""
    nc = bacc.Bacc(None)

    slab_d = nc.declare_dram_parameter("slab", [P, total], FP8, isOutput=False)
    out_d = nc.declare_dram_parameter("out", [P, ns * D], FP8, isOutput=True)

    # Chunk cuts in slots, growth-shaped like the measured-best feature
    # chunking ([1,2,3,4,4,3] slots); all chunks stream via GPSIMD SWDGE,
    # the SP HWDGE ring carries only the output drains.
    # (Splitting the tail into 2-slot chunks to drain the evac backlog
    # sooner measured neutral-to-worse: the extra receipt costs more.)
    cuts = sorted(set([c for c in (1, 3, 6, 10, 14) if c < ns] + [ns]))
    # first chunk starts at column 0 so the head iv block is covered
    bounds = [0] + [slotcols[c] for c in cuts]
    chunks = list(zip(bounds[:-1], bounds[1:]))
    ogroups = _ogroups(ns)

    with tile.TileContext(nc) as tc:
        with (
            tc.tile_pool(name="warm", bufs=1) as warm_pool,
            tc.tile_pool(name="mslab", bufs=1) as m_pool,
            tc.tile_pool(name="outp", bufs=1) as out_pool,
            tc.tile_pool(name="psum", bufs=4, space="PSUM") as psum_pool,
        ):
            # Warm-up source via VectorE so GpSimd's first op is the SWDGE
            # descriptor generation for chunk 0.
            wsrc = warm_pool.tile([P, 512], FP8)
            nc.vector.memset(wsrc[:], 0.25)

            # Chunk 0 rides the SP HWDGE ring: it is otherwise idle until
            # the first output drain (~17.7 us), and HWDGE's ~0.6 us
            # first-byte beats SWDGE's Q7 spool -- the first slot's data
            # lands ~0.5 us earlier.  (Earlier sync-ring feature attempts
            # failed only because masks shared that ring; they no longer do.)
            mask_sb = m_pool.tile([P, total], FP8)
            for j, (lo, hi) in enumerate(chunks):
                eng = nc.sync if j == 0 else nc.gpsimd
                eng.dma_start(out=mask_sb[:, lo:hi], in_=slab_d[:, lo:hi])

            # Pre-warm the ACT HWDGE ring with a tiny dummy transfer: the
            # ring's first DMA otherwise pays its spool-up inside the
            # FINAL out drain's descriptor-gen (1433 ns observed vs ~600).
            wdma = warm_pool.tile([P, 32], FP8)
            nc.scalar.dma_start(out=wdma[:], in_=slab_d[:, 0:32])

            # PE p-state warm-up: keep the PE busy until the first slot's
            # mask + feature DMAs land.
            wps = psum_pool.tile([P, 1024], F32, name="warm", tag="ps")
            for _ in range(N_WARM):
                nc.tensor.matmul(
                    wps[:, 0:512], wsrc[:, 0:P], wsrc[:], start=True, stop=True
                )

            os_slab = out_pool.tile([P, ns, D], FP8)
            gcuts = np.cumsum([0] + ogroups)
            gi = 0
            for s in range(ns):
                # [P, 1024] = exactly 2 PSUM banks per buf; 4 bufs fill all
                # 8 banks and give the evacuation two slots of slack.
                ps = psum_pool.tile([P, 1024], F32, name=f"ps{s}", tag="ps")
                np_s = len(pairs[s])
                for j, (cb, w, moff, fb) in enumerate(pairs[s]):
                    lh = mask_sb[:, cb : cb + w]
                    first = j == 0
                    last = j == np_s - 1
                    for n0, nn in ((0, 512), (512, 256)):
                        nc.tensor.matmul(
                            ps[moff : moff + w, n0 : n0 + nn],
                            lh, mask_sb[:, fb + n0 : fb + n0 + nn],
                            start=first, stop=(last and n0 == 512),
                        )
                # PSUM evacuation with the 1/count scale (read out of the
                # mask slab via a 4-byte fp8->fp32 bitcast); 384+384 balances
                # ScalarE (0.83 ns/el + 293 ns) vs VectorE (1.04 + 217).
                iv_s = mask_sb[:, ivbase + 4 * s : ivbase + 4 * s + 4].bitcast(
                    F32
                )
                nc.scalar.mul(
                    out=os_slab[:, s, 0:384], in_=ps[:, 0:384], mul=iv_s
                )
                nc.vector.tensor_scalar(
                    os_slab[:, s, 384:D], ps[:, 384:D],
                    iv_s, None, mybir.AluOpType.mult,
                )
                if s == gcuts[gi + 1] - 1:
                    g0, g1 = gcuts[gi], gcuts[gi + 1]
                    r = last_rows if g1 == ns and g1 - g0 == 1 else P
                    # The FINAL group rides the ACT HWDGE ring: on the SP
                    # ring its descriptor-gen queues behind the two
                    # preceding groups' gens (~1.6 us observed); on the
                    # scalar ring it fires right after the last evacuation,
                    # and its sem-wait can block nothing (no scalar ops
                    # follow it).
                    eng = nc.scalar if g1 == ns else nc.sync
                    eng.dma_start(
                        out=out_d[0:r, g0 * D : g1 * D],
                        in_=os_slab[0:r, g0:g1, :],
                    )
                    gi += 1

    nc.finalize()
    return nc


def _assign_slots(b, e_eff):
    """Per-core greedy slot assignment in sorted-begin order: a slot takes
    up to 128 windows whose token-interval union stays <= SLOT_TOK tokens.

    Returns (ns, slot_of[B,W], pos_of[B,W], slot_tokens[c][s] -> np.array).
    """
    slot_of = np.full((B, W), -1, np.int32)
    pos_of = np.full((B, W), -1, np.int32)
    slot_tokens = []
    ns = 0
    for c in range(B):
        order = np.argsort(b[c], kind="stable")
        bs, es = b[c][order], e_eff[c][order]
        toks_c = []
        i = 0
        while i < W:
            covered = 0
            cur_end = int(bs[i])
            ivals = []
            j = i
            while j < W and j - i < P:
                nb, ne = int(bs[j]), int(es[j])
                add = max(0, ne - max(nb, cur_end))
                if covered + add > SLOT_TOK:
                    break
                covered += add
                if ne > cur_end:
                    ivals.append((max(nb, cur_end), ne))
                    cur_end = ne
                j += 1
            s = len(toks_c)
            slot_of[c, order[i:j]] = s
            pos_of[c, order[i:j]] = np.arange(j - i)
            toks_c.append(
                np.concatenate([np.arange(a, z) for a, z in ivals])
                if ivals
                else np.zeros(0, np.int64)
            )
            i = j
        slot_tokens.append(toks_c)
        ns = max(ns, len(toks_c))
    return ns, slot_of, pos_of, slot_tokens


def _prepare(features, begins, ends):
    feats = np.asarray(features, dtype=np.float32)
    assert feats.shape == (B, T, D), feats.shape
    b = np.clip(np.asarray(begins).astype(np.int64), 0, T - 1)
    e = np.asarray(ends).astype(np.int64)
    # Reference gathers at most MAXWIN tokens starting at b; empty -> count 1.
    e_eff = np.clip(e, b, np.minimum(b + MAXWIN, T))
    counts = np.maximum(e_eff - b, 1).astype(np.float32)
    inv = (1.0 / counts).astype(np.float32)

    ns, slot_of, pos_of, slot_tokens = _assign_slots(b, e_eff)

    # Joint (cross-core max) pair structure per slot.  Windows are sorted by
    # begin, so windows starting in tile 0 are a position-prefix [0, c1) and
    # windows reaching into tile 1 are a position-suffix [c0, 128), c0 <= c1.
    # Ship only mask columns [0, c1r) for pair 0 and [c0r, 128) for pair 1
    # (32-aligned); the two matmuls' M-ranges still cover every PSUM row.
    lbs = [[None] * ns for _ in range(B)]
    les = [[None] * ns for _ in range(B)]
    c1s = np.zeros(ns, np.int64)
    c0s = np.full(ns, P, np.int64)
    two = np.zeros(ns, bool)  # slot has any tile-1 tokens on any core
    for c in range(B):
        for s, toks in enumerate(slot_tokens[c]):
            ws = np.nonzero(slot_of[c] == s)[0]
            if not len(ws):
                continue
            o = np.argsort(pos_of[c, ws])
            ws = ws[o]
            lb = np.searchsorted(toks, b[c, ws])
            le = lb + (e_eff[c, ws] - b[c, ws])
            lbs[c][s], les[c][s] = lb, le
            c1s[s] = max(c1s[s], int((lb < P).sum()))
            over = np.nonzero(le > P)[0]
            if len(over):
                c0s[s] = min(c0s[s], int(over[0]))
            if len(toks) > P:
                two[s] = True
    # A matmul PSUM write with a nonzero partition offset may span at most
    # one 32-partition col-group (walrus birverifier), so only pair 0's
    # column TAIL is trimmed (its windows are a position-prefix).  The
    # full-width tile-1 pair runs FIRST in each slot: its start=True write
    # covers (and has_written-clears) every PSUM row, and the trimmed
    # tile-0 pair then accumulates on rows [0, c1r) only.
    # Unified slab layout: per slot [tile1 | tile0 | mask1(full) |
    # mask0(trimmed)], with the fp32 1/count block once before slot IVSLOT.
    # Pair tuples: (mask col base, width, psum row offset, feature col base)
    pairs = []
    col = 0
    ivblk = -(-4 * ns // 32) * 32
    ivbase = -1
    slotcols = []
    for s in range(ns):
        if s == IVSLOT:
            ivbase = col
            col += ivblk
        slotcols.append(col)
        if two[s]:
            c1r = min(P, -(-int(c1s[s]) // 32) * 32)
            fb1, fb0 = col, col + D
            mc = col + 2 * D
            pairs.append([(mc, P, 0, fb1), (mc + P, c1r, 0, fb0)])
            col += 2 * D + P + c1r
        else:
            fb0 = col
            pairs.append([(fb0 + D, P, 0, fb0)])  # full width: every row
            col += D + P
    if ivbase < 0:
        ivbase = col
        col += ivblk
    slotcols.append(col)
    total = col

    in_maps = []
    unperm = []
    for c in range(B):
        slab = np.zeros((P, total), NP_FP8)
        ivm = np.zeros((P, ns), np.float32)
        ivm[pos_of[c], slot_of[c]] = inv[c]
        slab[:, ivbase : ivbase + 4 * ns] = ivm.view(NP_FP8)
        for s, toks in enumerate(slot_tokens[c]):
            nt = len(toks)
            # In the layout tile1 (tokens 128+) sits at the slot start, then
            # tile0: ascending feature col bases = [fb1, fb0].
            fbs = sorted(pr[3] for pr in pairs[s])
            if len(pairs[s]) == 2:
                fb1, fb0 = fbs[0], fbs[1]
            else:
                fb0, fb1 = fbs[0], None
            if nt:
                pk = feats[c, toks].astype(NP_FP8)
                h0 = np.zeros((P, D), NP_FP8)
                h0[: min(nt, P)] = pk[:P]
                slab[:, fb0 : fb0 + D] = h0
                if fb1 is not None:
                    h1 = np.zeros((P, D), NP_FP8)
                    if nt > P:
                        h1[: nt - P] = pk[P:]
                    slab[:, fb1 : fb1 + D] = h1
            if lbs[c][s] is None:
                continue
            lb, le = lbs[c][s], les[c][s]
            nw = len(lb)
            for cb, w, moff, fb in pairs[s]:
                tloc = 1 if fb == fb1 else 0  # which 128-token half
                lt = P * tloc + np.arange(P)  # local token row per partition
                sub = (
                    (lb[None, :] <= lt[:, None]) & (lt[:, None] < le[None, :])
                ).astype(NP_FP8)
                # columns for positions [moff, moff+w) of this slot
                pcols = np.arange(moff, min(moff + w, nw))
                slab[:, cb + pcols - moff] = sub[:, pcols]
        in_maps.append({"slab": slab})
        unperm.append((slot_of[c].astype(np.int64), pos_of[c].astype(np.int64)))
    last_rows = max(
        1, int((slot_of == ns - 1).sum(axis=1).max())
    )
    return ns, pairs, total, ivbase, slotcols, in_maps, unperm, last_rows


def run(features, begins, ends, trace=False):
    """Build + run on 8 NeuronCores; returns (output, BassKernelResults)."""
    ns, pairs, total, ivbase, slotcols, in_maps, unperm, last_rows = _prepare(
        features, begins, ends
    )
    nc = _build_program(ns, pairs, total, ivbase, slotcols, last_rows)
    res = run_bass_kernel_spmd(nc, in_maps, list(range(B)), trace=trace)
    out = np.stack(
        [
            res.results[c]["out"]
            .reshape(P, ns, D)[unperm[c][1], unperm[c][0]]
            .astype(np.float32)
            for c in range(B)
        ],
        axis=0,
    )
    return out, res


def kernel(features, begins, ends):
    out, _ = run(features, begins, ends, trace=False)
    return out
